# revision 22
# baseline (speedup 1.0000x reference)
"""DressedQuantumNet on 8 TRN2 NeuronCores (pure data parallel).

Math: pre-net angles th = X @ pre_w.T (+ pre_b).  After H + RY(th') the
4-qubit state is the REAL product state

  psi = kron_w [cos(th'_w/2 + pi/4), sin(th'_w/2 + pi/4)],  th' = th + pre_b

and the rest of the circuit is a FIXED unitary V (depends only on
q_weights).  The per-wire bias rotation R(pre_b_w/2) is absorbed into
V' = V @ kron_w R(pre_b_w/2), so the device only needs raw th.  With
Wz[i,c] = sum_w post_w[c,w] z_w(i) + post_b[c] (sum_i probs_i == 1), the
whole head collapses to two real symmetric quadratic forms

  out_c = psi^T K_c psi,   K_c = Re(V'^H diag(Wz[:,c]) V')   [16 x 16]

Device pipeline per 8-row-tile slab (batch on SBUF partitions), software
pipelined A(s) | B(s-1) | C(s-2) to keep all engine FIFOs unblocked:

  A: DMA fp16 X^T slab (1 MiB, sync queue; consts preloaded on same queue)
     PE  4 accumulating matmuls/tile -> th in PSUM [128,t,4]
     ACT cs = Sin(+-0.5*th + pi/4)  -> (cos,sin) fp16
     DVE psi = (c0,s0)x(c1,s1)x(c2,s2)x(c3,s3)  [128, t, 16] fp16
  B: PE  transpose psi -> psiT [16t, 128] PSUM; ACT copy -> SBUF
     PE  qq_c = blockdiag(K_c) @ psiT  (c=0,1 -> one PSUM tile)
  C: DVE pq_c = psiT * qq_c  (SBUF x PSUM -> fp16)
     PE  z[t, c, p] = column-sum over the 16 states (selection matmul)
     ACT copy z -> resall; one DMA out at the end

Everything sits under the fp16 input-stream DMA floor (~8.4 MiB/core).
"""

from contextlib import ExitStack

import numpy as np

import concourse.bass as bass
import concourse.bacc as bacc_mod
import concourse.mybir as mybir
from concourse.bass_utils import run_bass_kernel_spmd
from concourse.tile import TileContext

N_CORES = 8
B_TOTAL = 65536
F_IN = 512
ROWS = B_TOTAL // N_CORES   # 8192 rows per core
P = 128
N_TILES = ROWS // P         # 64 row-tiles
G = 8                       # row-tiles per slab (1 MiB fp16 DMA)
N_SLABS = N_TILES // G      # 8

F32 = mybir.dt.float32
FP16 = mybir.dt.float16
PI = float(np.pi)

N_QUBITS, VAR_DEPTH = 4, 3


# ----------------------------------------------------------------- host math
def _gate_1q(g, w):
    ops = [np.eye(2, dtype=complex)] * N_QUBITS
    ops[w] = g
    U = ops[0]
    for i in range(1, N_QUBITS):
        U = np.kron(U, ops[i])
    return U


def _bit(i, w):  # wire 0 = most significant
    return (i >> (N_QUBITS - 1 - w)) & 1


def _cnot(c, t):
    M = np.zeros((16, 16), dtype=complex)
    for i in range(16):
        j = i ^ (1 << (N_QUBITS - 1 - t)) if _bit(i, c) else i
        M[j, i] = 1.0
    return M


def _ry(theta):
    c, s = np.cos(theta / 2), np.sin(theta / 2)
    return np.array([[c, -s], [s, c]], dtype=complex)


def _rz(theta):
    ph = np.exp(1j * theta / 2)
    return np.array([[np.conj(ph), 0], [0, ph]], dtype=complex)


def _fixed_unitary(qw):
    V = np.eye(16, dtype=complex)

    def app(Gm):
        nonlocal V
        V = Gm @ V

    def entangle():
        app(_cnot(0, 1)); app(_cnot(2, 3)); app(_cnot(1, 2))

    for k in range(VAR_DEPTH):
        entangle()
        for w in range(N_QUBITS):
            app(_gate_1q(_ry(qw[k, w]), w))
        for w in range(N_QUBITS):
            app(_gate_1q(_rz(qw[k, w]), w))
    for k in range(VAR_DEPTH):
        entangle()
        for w in range(N_QUBITS):
            app(_gate_1q(_ry(qw[k, w]), w))
        for w in range(N_QUBITS):
            app(_gate_1q(_rz(qw[3 + k, w]), w))
    entangle()
    return V


def _host_consts(pre_w, pre_b, q_weights, post_w, post_b):
    pre_w = np.asarray(pre_w, dtype=np.float64)
    pre_b = np.asarray(pre_b, dtype=np.float64)
    post_w = np.asarray(post_w, dtype=np.float64)
    post_b = np.asarray(post_b, dtype=np.float64)

    # whl[p, 4k + w] = pre_w[w, 128k + p]
    whl = np.zeros((P, 16), dtype=np.float16)
    for k in range(4):
        whl[:, 4 * k:4 * k + 4] = pre_w.T[P * k:P * (k + 1)].astype(np.float16)

    V = _fixed_unitary(np.asarray(q_weights, dtype=np.float64))
    R = np.eye(1)
    for w in range(N_QUBITS):
        d = pre_b[w] / 2.0
        R = np.kron(R, np.array([[np.cos(d), -np.sin(d)],
                                 [np.sin(d), np.cos(d)]]))
    Vp = V @ R

    # Wz[i, c] = sum_w post_w[c,w] z_w(i) + post_b[c]  (sum_i probs_i == 1)
    Wz = np.zeros((16, 2))
    for c in range(2):
        for i in range(16):
            Wz[i, c] = sum(
                post_w[c, w] * (1.0 - 2.0 * _bit(i, w)) for w in range(N_QUBITS)
            ) + post_b[c]

    # K_c = Re(V'^H diag(Wz_c) V')  -- real symmetric 16x16; block-diagonal
    kb = []
    for c in range(2):
        Kc = (Vp.conj().T @ np.diag(Wz[:, c]) @ Vp).real
        blk = np.zeros((P, P), dtype=np.float16)
        for t in range(G):
            blk[16 * t:16 * t + 16, 16 * t:16 * t + 16] = Kc.T.astype(np.float16)
        kb.append(blk)

    selz = np.zeros((P, G), dtype=np.float16)
    for t in range(G):
        selz[16 * t:16 * t + 16, t] = 1.0

    ident = np.eye(P, dtype=np.float16)
    blob = np.concatenate([whl, kb[0], kb[1], selz, ident], axis=1)
    return {"cblob": np.ascontiguousarray(blob)}


# ------------------------------------------------------------- device kernel
SCHED = [8] * 7 + [4, 4]            # tiles per DMA slab (sum = 64)
BATCHES = [4, 4]                    # units per epilogue batch (sum = 8)


def build_bass(rows=ROWS):
    n_tiles = rows // P
    assert sum(SCHED) == n_tiles
    n_units = n_tiles // G

    nc = bacc_mod.Bacc(None, target_bir_lowering=False)
    ht_d = nc.dram_tensor("htp", [rows * 4 * P], FP16, kind="ExternalInput")
    cb_d = nc.dram_tensor("cblob", [P, 16 + 3 * P + G], FP16, kind="ExternalInput")
    # out_dev[t, c, u, p] = out[(u*G + t)*128 + p, c]; host unscrambles
    out_d = nc.dram_tensor("out", [G, 2, n_units, P], F32, kind="ExternalOutput")

    with TileContext(nc) as tc, ExitStack() as ctx:
        const = ctx.enter_context(tc.tile_pool(name="const", bufs=1))
        cblob = const.tile([P, 16 + 3 * P + G], FP16)
        nc.sync.dma_start(cblob, cb_d[:])
        whl = cblob[:, 0:16]
        k0b = cblob[:, 16:16 + P]
        k1b = cblob[:, 16 + P:16 + 2 * P]
        selz = cblob[:, 16 + 2 * P:16 + 2 * P + G]
        ident = cblob[:, 16 + 2 * P + G:16 + 3 * P + G]
        pi4 = const.tile([P, 1], F32)
        nc.vector.memset(pi4, PI / 4)

        n_slabs = len(SCHED)
        xp = ctx.enter_context(tc.tile_pool(name="xin", bufs=n_slabs))
        angp = ctx.enter_context(tc.tile_pool(name="angp", bufs=2, space="PSUM"))
        csp = ctx.enter_context(tc.tile_pool(name="csp", bufs=4))
        pp = ctx.enter_context(tc.tile_pool(name="pp", bufs=3))
        psip = ctx.enter_context(tc.tile_pool(name="psip", bufs=n_slabs))
        ptp = ctx.enter_context(tc.tile_pool(name="ptp", bufs=2, space="PSUM"))
        pts = ctx.enter_context(tc.tile_pool(name="pts", bufs=2))
        qqp = ctx.enter_context(tc.tile_pool(name="qqp", bufs=1, space="PSUM"))
        prp = ctx.enter_context(tc.tile_pool(name="prp", bufs=2))
        zp = ctx.enter_context(tc.tile_pool(name="zp", bufs=1, space="PSUM"))
        rp = ctx.enter_context(tc.tile_pool(name="res", bufs=1))

        resall = rp.tile([G, 2, n_units, P], F32)

        # ---------------- phase 1: stream (forward edges only) ----------
        def stage_a(s, g, base):
            gb = g * P
            ht = xp.tile([P, 4, gb], FP16, tag="ht")
            nc.sync.dma_start(
                ht,
                ht_d[base:base + P * 4 * gb].rearrange(
                    "(p k b) -> p k b", p=P, k=4),
            )
            ang = angp.tile([P, g, 4], F32, tag="ang")
            for t in range(g):
                bs = t * P
                for k in range(4):
                    nc.tensor.matmul(
                        ang[:, t, :],
                        ht[:, k, bs:bs + P],
                        whl[:, 4 * k:4 * k + 4],
                        start=(k == 0), stop=(k == 3),
                    )
            cs = csp.tile([P, g, 4, 2], FP16, tag="cs")
            nc.scalar.activation(
                cs[:, :, :, 0], ang, mybir.ActivationFunctionType.Sin,
                bias=pi4, scale=-0.5,
            )
            nc.scalar.activation(
                cs[:, :, :, 1], ang, mybir.ActivationFunctionType.Sin,
                bias=pi4, scale=0.5,
            )
            p01 = pp.tile([P, g, 2, 2], FP16, tag="p01")
            nc.vector.tensor_mul(
                p01,
                cs[:, :, 0, :].unsqueeze(3).broadcast_to([P, g, 2, 2]),
                cs[:, :, 1, :].unsqueeze(2).broadcast_to([P, g, 2, 2]),
            )
            p23 = pp.tile([P, g, 2, 2], FP16, tag="p23")
            nc.vector.tensor_mul(
                p23,
                cs[:, :, 2, :].unsqueeze(3).broadcast_to([P, g, 2, 2]),
                cs[:, :, 3, :].unsqueeze(2).broadcast_to([P, g, 2, 2]),
            )
            psi = psip.tile([P, g, 4, 4], FP16, tag="psi")
            nc.vector.tensor_mul(
                psi,
                p01.rearrange("p g a b -> p g (a b)")
                   .unsqueeze(3).broadcast_to([P, g, 4, 4]),
                p23.rearrange("p g a b -> p g (a b)")
                   .unsqueeze(2).broadcast_to([P, g, 4, 4]),
            )
            return psi

        psis = []
        tile0, base = 0, 0
        slab_spans = []
        for s, g in enumerate(SCHED):
            psis.append(stage_a(s, g, base))
            slab_spans.append((tile0, g))
            tile0 += g
            base += P * 4 * g * P

        # ---------------- phase 2: epilogue ----------------------------
        # unit -> [(slab, chunk_start_in_slab, toff_in_unit, len)]
        unit_src = {u: [] for u in range(n_units)}
        for s, (t0, g) in enumerate(slab_spans):
            t = t0
            while t < t0 + g:
                u, toff = t // G, t % G
                ln = min(G - toff, t0 + g - t)
                unit_src[u].append((s, t - t0, toff, ln))
                t += ln

        u0b = [0]
        for nu in BATCHES:
            u0b.append(u0b[-1] + nu)

        ptiles, ptss, qqs, pqs, zs = {}, {}, {}, {}, {}
        for b, nu in enumerate(BATCHES):
            ptile = ptp.tile([P, nu, P], FP16, tag="pt", name="ptile")
            for k in range(nu):
                u = u0b[b] + k
                for s, cs0, toff, ln in unit_src[u]:
                    nc.tensor.transpose(
                        ptile[16 * toff:16 * (toff + ln), k, :],
                        psis[s][:, cs0:cs0 + ln, :, :]
                            .rearrange("p g a b -> p (g a b)"),
                        ident,
                    )
            ptiles[b] = ptile
        for b, nu in enumerate(BATCHES):
            psiTs = pts.tile([P, nu, P], FP16, tag="psiTs", name="psiTs")
            nc.scalar.copy(psiTs, ptiles[b])
            ptss[b] = psiTs
        for b, nu in enumerate(BATCHES):
            qq = qqp.tile([P, 2, nu, P], F32, tag="qq", name="qq")
            rhs = ptss[b].rearrange("p u b -> p (u b)")
            nc.tensor.matmul(qq[:, 0].rearrange("p u b -> p (u b)"),
                             k0b, rhs, start=True, stop=True)
            nc.tensor.matmul(qq[:, 1].rearrange("p u b -> p (u b)"),
                             k1b, rhs, start=True, stop=True)
            qqs[b] = qq
        for b, nu in enumerate(BATCHES):
            pq = prp.tile([P, 2, nu, P], FP16, tag="pq", name="pq")
            nc.vector.tensor_mul(pq[:, 0], qqs[b][:, 0], ptss[b])
            nc.vector.tensor_mul(pq[:, 1], qqs[b][:, 1], ptss[b])
            pqs[b] = pq
        for b, nu in enumerate(BATCHES):
            z_ps = zp.tile([G, 2, nu, P], F32, tag="z", name="z_ps")
            for c in range(2):
                nc.tensor.matmul(
                    z_ps[:, c].rearrange("t u b -> t (u b)"), selz,
                    pqs[b][:, c].rearrange("p u b -> p (u b)"),
                    start=True, stop=True,
                )
            zs[b] = z_ps
        for b, nu in enumerate(BATCHES):
            nc.scalar.copy(resall[:, :, u0b[b]:u0b[b] + nu, :], zs[b])
            nc.gpsimd.dma_start(out_d[:, :, u0b[b]:u0b[b] + nu, :],
                                resall[:, :, u0b[b]:u0b[b] + nu, :])

    nc.finalize()
    return nc


_NC_CACHE = {}


def _get_nc(rows=ROWS):
    if rows not in _NC_CACHE:
        _NC_CACHE[rows] = build_bass(rows=rows)
    return _NC_CACHE[rows]


def _pack_input(x):
    """x [ROWS, F] f32 -> flat fp16: per-slab [P, 4, g*P] packs,
    pack[p, k, b] = x[slab_row0 + b, 128*k + p]."""
    h = x.astype(np.float16)
    parts = []
    r0 = 0
    for g in SCHED:
        gb = g * P
        blk = h[r0:r0 + gb].reshape(gb, 4, P).transpose(2, 1, 0)
        parts.append(np.ascontiguousarray(blk).reshape(-1))
        r0 += gb
    return np.concatenate(parts)


def run(input_features, pre_w, pre_b, q_weights, post_w, post_b, **spmd_kwargs):
    x = np.asarray(input_features, dtype=np.float32)
    assert x.shape == (B_TOTAL, F_IN), x.shape
    consts = _host_consts(pre_w, pre_b, q_weights, post_w, post_b)
    in_maps = []
    for c in range(N_CORES):
        ht = _pack_input(x[c * ROWS:(c + 1) * ROWS])
        in_maps.append(dict(consts, htp=ht))
    nc = _get_nc()
    r = run_bass_kernel_spmd(nc, in_maps, core_ids=list(range(N_CORES)), **spmd_kwargs)
    # out_dev[t, c, s, p] -> out[(s*G + t)*128 + p, c]
    outs = []
    for c in range(N_CORES):
        o = r.results[c]["out"]                             # [t, c, s, p]
        o = o.transpose(2, 0, 3, 1).reshape(ROWS, 2)        # [s, t, p, c]
        outs.append(o)
    out = np.concatenate(outs, axis=0)
    return out.astype(np.float32), r


def kernel(input_features, pre_w, pre_b, q_weights, post_w, post_b):
    out, _ = run(input_features, pre_w, pre_b, q_weights, post_w, post_b)
    return out


# revision 24
# speedup vs baseline: 1.0404x; 1.0404x over previous
"""DressedQuantumNet on 8 TRN2 NeuronCores (pure data parallel).

Math: pre-net angles th = X @ pre_w.T (+ pre_b).  After H + RY(th') the
4-qubit state is the REAL product state

  psi = kron_w [cos(th'_w/2 + pi/4), sin(th'_w/2 + pi/4)],  th' = th + pre_b

and the rest of the circuit is a FIXED unitary V (depends only on
q_weights).  The per-wire bias rotation R(pre_b_w/2) is absorbed into
V' = V @ kron_w R(pre_b_w/2), so the device only needs raw th.  With
Wz[i,c] = sum_w post_w[c,w] z_w(i) + post_b[c] (sum_i probs_i == 1), the
whole head collapses to two real symmetric quadratic forms

  out_c = psi^T K_c psi,   K_c = Re(V'^H diag(Wz[:,c]) V')   [16 x 16]

Device pipeline per 8-row-tile slab (batch on SBUF partitions), software
pipelined A(s) | B(s-1) | C(s-2) to keep all engine FIFOs unblocked:

  A: DMA fp16 X^T slab (1 MiB, sync queue; consts preloaded on same queue)
     PE  4 accumulating matmuls/tile -> th in PSUM [128,t,4]
     ACT cs = Sin(+-0.5*th + pi/4)  -> (cos,sin) fp16
     DVE psi = (c0,s0)x(c1,s1)x(c2,s2)x(c3,s3)  [128, t, 16] fp16
  B: PE  transpose psi -> psiT [16t, 128] PSUM; ACT copy -> SBUF
     PE  qq_c = blockdiag(K_c) @ psiT  (c=0,1 -> one PSUM tile)
  C: DVE pq_c = psiT * qq_c  (SBUF x PSUM -> fp16)
     PE  z[t, c, p] = column-sum over the 16 states (selection matmul)
     ACT copy z -> resall; one DMA out at the end

Everything sits under the fp16 input-stream DMA floor (~8.4 MiB/core).
"""

from contextlib import ExitStack

import numpy as np

import concourse.bass as bass
import concourse.bacc as bacc_mod
import concourse.mybir as mybir
from concourse.bass_utils import run_bass_kernel_spmd
from concourse.tile import TileContext

N_CORES = 8
B_TOTAL = 65536
F_IN = 512
ROWS = B_TOTAL // N_CORES   # 8192 rows per core
P = 128
N_TILES = ROWS // P         # 64 row-tiles
G = 8                       # row-tiles per slab (1 MiB fp16 DMA)
N_SLABS = N_TILES // G      # 8

F32 = mybir.dt.float32
FP16 = mybir.dt.float16
PI = float(np.pi)

N_QUBITS, VAR_DEPTH = 4, 3


# ----------------------------------------------------------------- host math
def _gate_1q(g, w):
    ops = [np.eye(2, dtype=complex)] * N_QUBITS
    ops[w] = g
    U = ops[0]
    for i in range(1, N_QUBITS):
        U = np.kron(U, ops[i])
    return U


def _bit(i, w):  # wire 0 = most significant
    return (i >> (N_QUBITS - 1 - w)) & 1


def _cnot(c, t):
    M = np.zeros((16, 16), dtype=complex)
    for i in range(16):
        j = i ^ (1 << (N_QUBITS - 1 - t)) if _bit(i, c) else i
        M[j, i] = 1.0
    return M


def _ry(theta):
    c, s = np.cos(theta / 2), np.sin(theta / 2)
    return np.array([[c, -s], [s, c]], dtype=complex)


def _rz(theta):
    ph = np.exp(1j * theta / 2)
    return np.array([[np.conj(ph), 0], [0, ph]], dtype=complex)


def _fixed_unitary(qw):
    V = np.eye(16, dtype=complex)

    def app(Gm):
        nonlocal V
        V = Gm @ V

    def entangle():
        app(_cnot(0, 1)); app(_cnot(2, 3)); app(_cnot(1, 2))

    for k in range(VAR_DEPTH):
        entangle()
        for w in range(N_QUBITS):
            app(_gate_1q(_ry(qw[k, w]), w))
        for w in range(N_QUBITS):
            app(_gate_1q(_rz(qw[k, w]), w))
    for k in range(VAR_DEPTH):
        entangle()
        for w in range(N_QUBITS):
            app(_gate_1q(_ry(qw[k, w]), w))
        for w in range(N_QUBITS):
            app(_gate_1q(_rz(qw[3 + k, w]), w))
    entangle()
    return V


def _host_consts(pre_w, pre_b, q_weights, post_w, post_b):
    pre_w = np.asarray(pre_w, dtype=np.float64)
    pre_b = np.asarray(pre_b, dtype=np.float64)
    post_w = np.asarray(post_w, dtype=np.float64)
    post_b = np.asarray(post_b, dtype=np.float64)

    # whl[p, 4k + w] = pre_w[w, 128k + p]
    whl = np.zeros((P, 16), dtype=np.float16)
    for k in range(4):
        whl[:, 4 * k:4 * k + 4] = pre_w.T[P * k:P * (k + 1)].astype(np.float16)

    V = _fixed_unitary(np.asarray(q_weights, dtype=np.float64))
    R = np.eye(1)
    for w in range(N_QUBITS):
        d = pre_b[w] / 2.0
        R = np.kron(R, np.array([[np.cos(d), -np.sin(d)],
                                 [np.sin(d), np.cos(d)]]))
    Vp = V @ R

    # Wz[i, c] = sum_w post_w[c,w] z_w(i) + post_b[c]  (sum_i probs_i == 1)
    Wz = np.zeros((16, 2))
    for c in range(2):
        for i in range(16):
            Wz[i, c] = sum(
                post_w[c, w] * (1.0 - 2.0 * _bit(i, w)) for w in range(N_QUBITS)
            ) + post_b[c]

    # K_c = Re(V'^H diag(Wz_c) V')  -- real symmetric 16x16; block-diagonal
    kb = []
    for c in range(2):
        Kc = (Vp.conj().T @ np.diag(Wz[:, c]) @ Vp).real
        blk = np.zeros((P, P), dtype=np.float16)
        for t in range(G):
            blk[16 * t:16 * t + 16, 16 * t:16 * t + 16] = Kc.T.astype(np.float16)
        kb.append(blk)

    selz = np.zeros((P, G), dtype=np.float16)
    for t in range(G):
        selz[16 * t:16 * t + 16, t] = 1.0

    ident = np.eye(P, dtype=np.float16)
    blob = np.concatenate([whl, kb[0], kb[1], selz, ident], axis=1)
    return {"cblob": np.ascontiguousarray(blob)}


# ------------------------------------------------------------- device kernel
SCHED = [8, 16, 16, 16, 4, 4]       # tiles per DMA slab (sum = 64)
BATCHES = [4, 2, 2]                 # units per epilogue batch (sum = 8)


def build_bass(rows=ROWS):
    n_tiles = rows // P
    assert sum(SCHED) == n_tiles
    n_units = n_tiles // G

    nc = bacc_mod.Bacc(None, target_bir_lowering=False)
    ht_d = nc.dram_tensor("htp", [rows * 4 * P], FP16, kind="ExternalInput")
    cb_d = nc.dram_tensor("cblob", [P, 16 + 3 * P + G], FP16, kind="ExternalInput")
    # out_dev[t, c, u, p] = out[(u*G + t)*128 + p, c]; host unscrambles
    out_d = nc.dram_tensor("out", [G, 2, n_units, P], F32, kind="ExternalOutput")

    with TileContext(nc) as tc, ExitStack() as ctx:
        const = ctx.enter_context(tc.tile_pool(name="const", bufs=1))
        cblob = const.tile([P, 16 + 3 * P + G], FP16)
        nc.sync.dma_start(cblob, cb_d[:])
        whl = cblob[:, 0:16]
        k0b = cblob[:, 16:16 + P]
        k1b = cblob[:, 16 + P:16 + 2 * P]
        selz = cblob[:, 16 + 2 * P:16 + 2 * P + G]
        ident = cblob[:, 16 + 2 * P + G:16 + 3 * P + G]
        pi4 = const.tile([P, 1], F32)
        nc.vector.memset(pi4, PI / 4)

        n_slabs = len(SCHED)
        xp = ctx.enter_context(tc.tile_pool(name="xin", bufs=n_slabs))
        angp = ctx.enter_context(tc.tile_pool(name="angp", bufs=2, space="PSUM"))
        csp = ctx.enter_context(tc.tile_pool(name="csp", bufs=4))
        pp = ctx.enter_context(tc.tile_pool(name="pp", bufs=3))
        psip = ctx.enter_context(tc.tile_pool(name="psip", bufs=4))
        ptp = ctx.enter_context(tc.tile_pool(name="ptp", bufs=2, space="PSUM"))
        pts = ctx.enter_context(tc.tile_pool(name="pts", bufs=2))
        qqp = ctx.enter_context(tc.tile_pool(name="qqp", bufs=1, space="PSUM"))
        prp = ctx.enter_context(tc.tile_pool(name="prp", bufs=2))
        zp = ctx.enter_context(tc.tile_pool(name="zp", bufs=1, space="PSUM"))
        rp = ctx.enter_context(tc.tile_pool(name="res", bufs=1))

        resall = rp.tile([G, 2, n_units, P], F32)

        def stage_a(s, g, base):
            gb = g * P
            ht = xp.tile([P, 4, gb], FP16, tag="ht")
            nc.sync.dma_start(
                ht,
                ht_d[base:base + P * 4 * gb].rearrange(
                    "(p k b) -> p k b", p=P, k=4),
            )
            ang = angp.tile([P, g, 4], F32, tag="ang")
            for t in range(g):
                bs = t * P
                for k in range(4):
                    nc.tensor.matmul(
                        ang[:, t, :],
                        ht[:, k, bs:bs + P],
                        whl[:, 4 * k:4 * k + 4],
                        start=(k == 0), stop=(k == 3),
                    )
            cs = csp.tile([P, g, 4, 2], FP16, tag="cs")
            nc.scalar.activation(
                cs[:, :, :, 0], ang, mybir.ActivationFunctionType.Sin,
                bias=pi4, scale=-0.5,
            )
            nc.scalar.activation(
                cs[:, :, :, 1], ang, mybir.ActivationFunctionType.Sin,
                bias=pi4, scale=0.5,
            )
            p01 = pp.tile([P, g, 2, 2], FP16, tag="p01")
            nc.vector.tensor_mul(
                p01,
                cs[:, :, 0, :].unsqueeze(3).broadcast_to([P, g, 2, 2]),
                cs[:, :, 1, :].unsqueeze(2).broadcast_to([P, g, 2, 2]),
            )
            p23 = pp.tile([P, g, 2, 2], FP16, tag="p23")
            nc.vector.tensor_mul(
                p23,
                cs[:, :, 2, :].unsqueeze(3).broadcast_to([P, g, 2, 2]),
                cs[:, :, 3, :].unsqueeze(2).broadcast_to([P, g, 2, 2]),
            )
            psi = psip.tile([P, g, 4, 4], FP16, tag="psi")
            nc.vector.tensor_mul(
                psi,
                p01.rearrange("p g a b -> p g (a b)")
                   .unsqueeze(3).broadcast_to([P, g, 4, 4]),
                p23.rearrange("p g a b -> p g (a b)")
                   .unsqueeze(2).broadcast_to([P, g, 4, 4]),
            )
            return psi

        # batch/unit layout
        batch_of_unit, ucol_of_unit, u0_of_batch = {}, {}, {}
        u = 0
        for b, nu in enumerate(BATCHES):
            u0_of_batch[b] = u
            for k in range(nu):
                batch_of_unit[u] = b
                ucol_of_unit[u] = k
                u += 1
        slab_info = []
        tile0, base = 0, 0
        for si, g in enumerate(SCHED):
            chunks = []
            t = tile0
            while t < tile0 + g:
                unit, toff = t // G, t % G
                ln = min(G - toff, tile0 + g - t)
                chunks.append((unit, toff, t - tile0, ln))
                t += ln
            slab_info.append((g, base, chunks))
            tile0 += g
            base += P * 4 * g * P
        units_left = {b: nu for b, nu in enumerate(BATCHES)}

        ptiles = {}

        def transpose_slab(si):
            g, base, chunks = slab_info[si]
            psi = psis.pop(si)
            done = []
            for unit, toff, cs0, ln in chunks:
                b = batch_of_unit[unit]
                if b not in ptiles:
                    ptiles[b] = ptp.tile([P, BATCHES[b], P], FP16, tag="pt",
                                         name="ptile")
                nc.tensor.transpose(
                    ptiles[b][16 * toff:16 * (toff + ln), ucol_of_unit[unit], :],
                    psi[:, cs0:cs0 + ln, :, :].rearrange("p g a b -> p (g a b)"),
                    ident,
                )
                if toff + ln == G:
                    units_left[b] -= 1
                    if units_left[b] == 0:
                        done.append(b)
            return done

        def epilogues(blist):
            work = []
            for b in blist:
                nu, u0 = BATCHES[b], u0_of_batch[b]
                work.append((b, nu, u0, ptiles.pop(b)))
            pts_t, qq_t, pq_t, z_t = {}, {}, {}, {}
            for b, nu, u0, ptile in work:
                psiTs = pts.tile([P, nu, P], FP16, tag="psiTs", name="psiTs")
                nc.scalar.copy(psiTs, ptile)
                pts_t[b] = psiTs
            for b, nu, u0, ptile in work:
                qq = qqp.tile([P, 2, nu, P], F32, tag="qq", name="qq")
                rhs = pts_t[b].rearrange("p u b -> p (u b)")
                nc.tensor.matmul(qq[:, 0].rearrange("p u b -> p (u b)"),
                                 k0b, rhs, start=True, stop=True)
                nc.tensor.matmul(qq[:, 1].rearrange("p u b -> p (u b)"),
                                 k1b, rhs, start=True, stop=True)
                qq_t[b] = qq
            for b, nu, u0, ptile in work:
                pq = prp.tile([P, 2, nu, P], FP16, tag="pq", name="pq")
                nc.vector.tensor_mul(pq[:, 0], qq_t[b][:, 0], pts_t[b])
                nc.vector.tensor_mul(pq[:, 1], qq_t[b][:, 1], pts_t[b])
                pq_t[b] = pq
            for b, nu, u0, ptile in work:
                z_ps = zp.tile([G, 2, nu, P], F32, tag="z", name="z_ps")
                for c in range(2):
                    nc.tensor.matmul(
                        z_ps[:, c].rearrange("t u b -> t (u b)"), selz,
                        pq_t[b][:, c].rearrange("p u b -> p (u b)"),
                        start=True, stop=True,
                    )
                z_t[b] = z_ps
            for b, nu, u0, ptile in work:
                nc.scalar.copy(resall[:, :, u0:u0 + nu, :], z_t[b])
            u0 = min(u0_of_batch[b] for b in blist)
            u1 = max(u0_of_batch[b] + BATCHES[b] for b in blist)
            nc.gpsimd.dma_start(out_d[:, :, u0:u1, :], resall[:, :, u0:u1, :])

        psis = {}
        n_slabs2 = len(SCHED)
        TL = 2                      # transpose emission lag (slabs)
        epi_q = []
        for s in range(n_slabs2 + TL):
            if epi_q:
                epilogues(epi_q)
                epi_q = []
            if s < n_slabs2:
                g, base, chunks = slab_info[s]
                psis[s] = stage_a(s, g, base)
            if s >= TL:
                epi_q.extend(transpose_slab(s - TL))
        if epi_q:
            epilogues(epi_q)

    nc.finalize()
    return nc


_NC_CACHE = {}


def _get_nc(rows=ROWS):
    if rows not in _NC_CACHE:
        _NC_CACHE[rows] = build_bass(rows=rows)
    return _NC_CACHE[rows]


def _pack_input(x):
    """x [ROWS, F] f32 -> flat fp16: per-slab [P, 4, g*P] packs,
    pack[p, k, b] = x[slab_row0 + b, 128*k + p]."""
    h = x.astype(np.float16)
    parts = []
    r0 = 0
    for g in SCHED:
        gb = g * P
        blk = h[r0:r0 + gb].reshape(gb, 4, P).transpose(2, 1, 0)
        parts.append(np.ascontiguousarray(blk).reshape(-1))
        r0 += gb
    return np.concatenate(parts)


def run(input_features, pre_w, pre_b, q_weights, post_w, post_b, **spmd_kwargs):
    x = np.asarray(input_features, dtype=np.float32)
    assert x.shape == (B_TOTAL, F_IN), x.shape
    consts = _host_consts(pre_w, pre_b, q_weights, post_w, post_b)
    in_maps = []
    for c in range(N_CORES):
        ht = _pack_input(x[c * ROWS:(c + 1) * ROWS])
        in_maps.append(dict(consts, htp=ht))
    nc = _get_nc()
    r = run_bass_kernel_spmd(nc, in_maps, core_ids=list(range(N_CORES)), **spmd_kwargs)
    # out_dev[t, c, s, p] -> out[(s*G + t)*128 + p, c]
    outs = []
    for c in range(N_CORES):
        o = r.results[c]["out"]                             # [t, c, s, p]
        o = o.transpose(2, 0, 3, 1).reshape(ROWS, 2)        # [s, t, p, c]
        outs.append(o)
    out = np.concatenate(outs, axis=0)
    return out.astype(np.float32), r


def kernel(input_features, pre_w, pre_b, q_weights, post_w, post_b):
    out, _ = run(input_features, pre_w, pre_b, q_weights, post_w, post_b)
    return out
